# revision 45
# baseline (speedup 1.0000x reference)
"""DeepseekV2 MLA decoder-layer attention on 8 Trainium2 NeuronCores.

Distribution (tensor-parallel over heads, per the source hint):
  - A-projection (hidden @ w_qkv_a.T) is sequence-sharded: each core computes
    the fused low-rank latents for its 256-token shard.  The kv latents are
    rmsnorm'd locally; the q latents ship RAW and their rsqrt row rides in the
    same payload (folded into the q B-projection epilogue after the gather).
    ONE AllGather moves everything -- measurements show each collective has
    ~20us fixed latency on top of ~1.8us/MB wire time, so a single gather
    beats any split schedule.
  - B-projections, RoPE(q_pe), flash-style causal attention and o_proj are
    head-sharded: core c owns heads {2c, 2c+1}; its o_proj against the matching
    w_o column slice yields a partial [2048, 2048] output.
  - Unshard on host: output = sum of the 8 partials (RowParallel reduction).

Schedule notes:
  - Phase-1 sum-of-squares matmuls are emitted one chunk late so the PE never
    waits on the scalar engine's square of the chunk it just finished.
  - The q-rope B-projection is packed across the core's two heads into one
    128-row matmul; k_pe is duplicated into both partition halves so the
    flash rope-score matmuls run at base partition 64*h.
  - Flash normalization for unit i is deferred until unit i+1's first three
    score matmuls are queued; o_proj tile-groups are drained one-per-consume
    into the flash stream, spreading the output DMA across the tail.
  - Diagonal score tiles only compute the causal column range.
"""
import ml_dtypes
import numpy as np

import concourse.bass as bass
import concourse.mybir as mybir
import concourse.tile as tile
from concourse import bacc
from concourse.bass_utils import run_bass_kernel_spmd

HIDDEN = 2048
H = 16
NOPE = 128
ROPE = 64
VDIM = 128
QLR = 1536
KVLR = 512
QK = NOPE + ROPE            # 192
THETA = 10000.0
EPS = 1e-6
SEQ = 2048

N_CORES = 8
HPC = H // N_CORES          # 2 heads per core
SSH = SEQ // N_CORES        # 256-token shard
LAT_C = 17                  # latent chunks: 12 q_a + 4 kv_a + 1 (kpe, 64 rows)
P = 128

F32 = mybir.dt.float32
F32R = mybir.dt.float32r
BF16 = mybir.dt.bfloat16
F16 = mybir.dt.float16
F8 = mybir.dt.float8e4
FLASH_DT = F16              # dtype of q/k/v/exp inside flash attention
LAT_DT = F16                # dtype of the AG1 payload + kv B-proj operands
EXP_BIAS = -4.0             # exp(x*scale + EXP_BIAS): cancels in softmax ratio,
                            # keeps fp16 exp values in range
WQ_SCALE = 32.0             # w_q_b pre-scaled into fp8's sweet spot on host;
                            # divided back out in the exp scale

SCALE = float(QK) ** -0.5
NEG = -1.0e30

N_KC = HIDDEN // P          # 16
N_QAC = QLR // P            # 12
N_KVC = KVLR // P           # 4
N_SB = SEQ // 512           # 4 query blocks
N_SC = SEQ // P             # 16
AG2_C = N_QAC + 1           # raw q(12) + rsqrt-row(1), fp16


def build_program():
    nc = bacc.Bacc("TRN2", target_bir_lowering=False, debug=False,
                   num_devices=N_CORES)

    h1 = nc.dram_tensor("h1", [P, N_KC, SSH], F16, kind="ExternalInput")
    w1 = nc.dram_tensor("w1", [LAT_C, P, HIDDEN], F16, kind="ExternalInput")
    wq = nc.dram_tensor("wq", [P, N_QAC, HPC * QK], LAT_DT, kind="ExternalInput")
    wkv = nc.dram_tensor("wkv", [P, N_KVC, HPC * (NOPE + VDIM)], LAT_DT, kind="ExternalInput")
    wo = nc.dram_tensor("wo", [P, HPC, HIDDEN], F16, kind="ExternalInput")
    cosq = nc.dram_tensor("cosq", [P, SEQ], F32R, kind="ExternalInput")
    ssinq = nc.dram_tensor("ssinq", [P, SEQ], F32R, kind="ExternalInput")
    cosl = nc.dram_tensor("cosl", [ROPE, SSH], F32R, kind="ExternalInput")
    ssinl = nc.dram_tensor("ssinl", [ROPE, SSH], F32R, kind="ExternalInput")
    pswap = nc.dram_tensor("pswap", [P, P], F32R, kind="ExternalInput")
    tri_d = nc.dram_tensor("tri", [P, P], F32, kind="ExternalInput")
    onesc_d = nc.dram_tensor("onesc", [P, 1], F32R, kind="ExternalInput")
    onesr_d = nc.dram_tensor("onesr", [1, P], F32R, kind="ExternalInput")
    yout = nc.dram_tensor("y", [SEQ, HIDDEN], F16, kind="ExternalOutput")

    with tile.TileContext(nc) as tc:
        _emit(nc, tc, h1, w1, wq, wkv, wo, cosq, ssinq, cosl, ssinl, pswap,
              tri_d, onesc_d, onesr_d, yout)
    nc.compile()
    return nc


def _emit(nc, tc, h1, w1, wq, wkv, wo, cosq, ssinq, cosl, ssinl, pswap,
          tri_d, onesc_d, onesr_d, yout):
    Exp = mybir.ActivationFunctionType.Exp
    Sqrt = mybir.ActivationFunctionType.Sqrt
    rg = [list(range(N_CORES))]

    with tc.tile_pool(name="const", bufs=1) as const, \
         tc.tile_pool(name="work", bufs=2) as work, \
         tc.tile_pool(name="lstr", bufs=4) as lstr, \
         tc.tile_pool(name="epool", bufs=5) as epool, \
         tc.tile_pool(name="big", bufs=1) as big, \
         tc.tile_pool(name="psum", bufs=1, space="PSUM") as psum, \
         tc.tile_pool(name="dram", bufs=1, space="DRAM") as dram:

        # PSUM tags (8 banks): qacc x3, oacc, oacc2, zacc, zacc2, bcast
        def ps(shape, tag, name, bufs=None):
            return psum.tile(shape, F32, tag=tag, name=name, bufs=bufs)

        # ---- first critical DMAs: operands of the very first matmuls ----
        ph1_cm = tc.tile_pool(name="ph1", bufs=1)
        ph1 = ph1_cm.__enter__()
        hloc = ph1.tile([P, N_KC, SSH], F16)
        wt0 = ph1.tile([P, HIDDEN], F16, name="w1t", bufs=3)
        nc.sync.dma_start(wt0[:, 0:512], w1[16, :, 0:512])
        nc.sync.dma_start(hloc[:, 0, :], h1[:, 0, :])
        ones_col_t = const.tile([P, 1], F32R)
        nc.sync.dma_start(ones_col_t[:], onesc_d[:])
        ones_col = ones_col_t[:]
        nc.sync.dma_start(wt0[:, 512:], w1[16, :, 512:])
        for k in range(1, N_KC):
            nc.sync.dma_start(hloc[:, k, :], h1[:, k, :])
        ones_row_t = const.tile([1, P], F32R)
        nc.sync.dma_start(ones_row_t[:], onesr_d[:])
        ones_row = ones_row_t[:]
        psw = const.tile([P, P], F32R)
        nc.sync.dma_start(psw[:], pswap[:])
        cl = const.tile([ROPE, 2, SSH], F32R)
        nc.sync.dma_start(cl[:, 0, :], cosl[:])
        nc.sync.dma_start(cl[:, 1, :], ssinl[:])
        eps1 = const.tile([1, 1], F32)
        nc.vector.memset(eps1[:], EPS)
        negc = const.tile([P, 1], F32)
        nc.vector.memset(negc[:], EXP_BIAS)
        rpad = const.tile([P, SSH], F16)
        nc.vector.memset(rpad[:], 0.0)

        # ---- phase-3 tiles (declared early; DMAs staggered into phase 1) ----
        lat = ph1.tile([P, 5, SSH], F32R)       # kv chunks 12..15 + kpe
        latf = ph1.tile([P, 5, SSH], LAT_DT)    # normalized kv + kpe, fp16
        latq8 = ph1.tile([P, N_QAC, SSH], LAT_DT)  # raw q latents, fp16
        wqs = big.tile([P, N_QAC, HPC * QK], LAT_DT)
        wkvs = big.tile([P, N_KVC, HPC * (NOPE + VDIM)], LAT_DT)
        wot = big.tile([P, HPC, HIDDEN], F16)
        tri = const.tile([P, P], F32)
        kn = [big.tile([P, SEQ], FLASH_DT, name=f"kn{h}") for h in range(HPC)]
        qn = [big.tile([P, SEQ], FLASH_DT, name=f"qn{h}") for h in range(HPC)]
        qpb2 = big.tile([P, SEQ], FLASH_DT)
        kpe2 = big.tile([P, N_CORES, SSH], LAT_DT)
        kv_all = big.tile([P, N_CORES, 5 * SSH], LAT_DT)
        vv = big.tile([P, N_SC, HPC * VDIM], FLASH_DT)
        op3 = big.tile([P, 16, 512], F16)       # block-3 o_proj h0 partials
        ao = [big.tile([P, SEQ], F16, name=f"ao{h}") for h in range(HPC)]
        onesc_f = const.tile([P, 1], FLASH_DT)
        with nc.allow_low_precision(reason="ones vector"):
            nc.vector.tensor_copy(onesc_f[:], ones_col)

        ss_q = ps([1, SSH], "zacc", "ss_q")
        ss_kv = ps([1, SSH], "zacc2", "ss_kv")

        def kv_norm_chain():
            rt = work.tile([1, SSH], F32, name="rtkv", bufs=1)
            nc.scalar.activation(rt[:], ss_kv[:], Sqrt, bias=eps1[:], scale=1.0 / KVLR)
            ri = work.tile([1, SSH], F32, name="rikv", bufs=1)
            nc.vector.reciprocal_approx_fast(ri[:], rt[:])
            rir = work.tile([1, SSH], F32R, name="rikvr", bufs=1)
            with nc.allow_low_precision(reason="float32r rounding"):
                nc.scalar.copy(rir[:], ri[:])
            bc = ps([P, SSH], "bcast", "bckv")
            nc.tensor.matmul(bc[:], ones_row[:], rir[:], start=True, stop=True)
            bcs = work.tile([P, SSH], F32, name="bcskv", bufs=1)
            nc.vector.tensor_copy(bcs[:], bc[:])
            for m in range(N_KVC):
                with nc.allow_low_precision(reason="fp16 AllGather payload"):
                    nc.vector.tensor_mul(latf[:, m, :], lat[:, m, :], bcs[:])

        ag_in1 = dram.tile([P, 5 * SSH], LAT_DT)
        ag_out1 = dram.tile([N_CORES, P, 5 * SSH], LAT_DT, addr_space="Shared")
        ag_in2 = dram.tile([P, AG2_C * SSH], LAT_DT)
        ag_out2 = dram.tile([N_CORES, P, AG2_C * SSH], LAT_DT, addr_space="Shared")

        # ================= phase 1: local A-proj =================
        # kv chunks first (12..15), then kpe (16), then raw q (0..11).
        # ss matmuls are deferred one chunk so the PE never waits on the
        # scalar square of the chunk it just finished.
        m_order = [16] + list(range(N_QAC, 16)) + list(range(N_QAC))
        pending_ss = [None]

        def drain_ss():
            if pending_ss[0] is not None:
                m_, sq_ = pending_ss[0]
                pending_ss[0] = None
                tgt = ss_q if m_ < N_QAC else ss_kv
                nc.tensor.matmul(tgt[:], ones_col[:], sq_[:],
                                 start=(m_ == 0) or (m_ == N_QAC),
                                 stop=(m_ == N_QAC - 1) or (m_ == 15))

        for m in m_order:
            acc = ps([P, SSH], "qacc", "a_acc", bufs=3)
            if m == m_order[0]:
                wt = wt0
            else:
                wt = ph1.tile([P, HIDDEN], F16, name="w1t", bufs=3)
                nc.sync.dma_start(wt[:], w1[m])
            for k in range(N_KC):
                nc.tensor.matmul(acc[:], wt[:, k * P:(k + 1) * P], hloc[:, k, :],
                                 start=(k == 0), stop=(k == N_KC - 1))
                if k == 2:
                    drain_ss()
            if m >= N_QAC:
                nc.vector.tensor_copy(lat[:, m - N_QAC, :], acc[:])
            else:
                with nc.allow_low_precision(reason="raw q latents ship fp16"):
                    nc.vector.tensor_copy(latq8[:, m, :], acc[:])
            if m < 16:
                sq = work.tile([P, SSH], F32R, name="sq", bufs=3)
                nc.scalar.square(sq[:], acc[:])
                if m == 15:
                    # emit immediately: the kv rmsnorm and AG1 trigger follow
                    drain_ss()
                    pending_ss[0] = (m, sq)
                    drain_ss()
                else:
                    pending_ss[0] = (m, sq)

            if m == 7:
                nc.sync.dma_start(wkvs[:], wkv[:])
            if m in (8, 9, 10):
                # phase-3 weight prefetch, late in phase 1 so the critical
                # early weight stream is undisturbed
                j = m - 8
                nc.sync.dma_start(wqs[:, 4 * j:4 * (j + 1), :],
                                  wq[:, 4 * j:4 * (j + 1), :])
            if m == 11:
                nc.sync.dma_start(wot[:], wo[:])
                nc.sync.dma_start(tri[:], tri_d[:])
            if m == 16:
                # rope on local k_pe (chunk 16 = lat idx 4, rows 0:64)
                swp = ps([ROPE, SSH], "qacc", "swp", bufs=3)
                nc.tensor.matmul(swp[:], psw[:ROPE, :ROPE], lat[:ROPE, 4, :],
                                 start=True, stop=True)
                sws = work.tile([ROPE, SSH], F32R, name="sws", bufs=1)
                nc.vector.tensor_mul(sws[:], swp[:], cl[:, 1, :])
                t1 = work.tile([ROPE, SSH], F32R, name="t1", bufs=1)
                nc.vector.tensor_mul(t1[:], lat[:ROPE, 4, :], cl[:, 0, :])
                with nc.allow_low_precision(reason="fp16 AllGather payload"):
                    nc.vector.tensor_add(latf[:ROPE, 4, :], t1[:], sws[:])
                    nc.vector.memset(latf[ROPE:, 4, :], 0.0)
            if m == 15:
                kv_norm_chain()
                # ---- AG1 (kv + kpe latents) overlaps the q-chunk matmuls
                nc.sync.dma_start(
                    ag_in1[:], latf[:].rearrange("p m s -> p (m s)"))
                nc.gpsimd.collective_compute(
                    "AllGather", mybir.AluOpType.bypass, replica_groups=rg,
                    ins=[ag_in1.opt()], outs=[ag_out1.opt()],
                )
            if m == 5:
                nc.sync.dma_start(
                    ag_in2[:, :6 * SSH],
                    latq8[:, 0:6, :].rearrange("p m s -> p (m s)"))

        drain_ss()
        # q rsqrt row (normalization folded in after the gather)
        rt_q = work.tile([1, SSH], F32, name="rtq", bufs=1)
        nc.scalar.activation(rt_q[:], ss_q[:], Sqrt, bias=eps1[:], scale=1.0 / QLR)
        ri_q = work.tile([1, SSH], F32, name="riq", bufs=1)
        nc.vector.reciprocal_approx_fast(ri_q[:], rt_q[:])
        with nc.allow_low_precision(reason="fp16 AllGather payload"):
            nc.vector.tensor_copy(rpad[0:1, :], ri_q[:])
        nc.sync.dma_start(
            ag_in2[:, 6 * SSH:12 * SSH],
            latq8[:, 6:12, :].rearrange("p m s -> p (m s)"))
        nc.sync.dma_start(ag_in2[:, 12 * SSH:], rpad[:])
        nc.gpsimd.collective_compute(
            "AllGather", mybir.AluOpType.bypass, replica_groups=rg,
            ins=[ag_in2.opt()], outs=[ag_out2.opt()],
        )
        # gathered views: agv1 idx 0..3 = kv, 4 = kpe; agv2 0..11 = raw q,
        # 12 = rsqrt row
        agv1 = ag_out1[:].rearrange("c p (m s) -> c p m s", m=5)
        agv2 = ag_out2[:].rearrange("c p (m s) -> c p m s", m=AG2_C)
        kpeA = kpe2[:].rearrange("p c s -> p (c s)")
        ph1_cm.__exit__(None, None, None)
        att_cm = tc.tile_pool(name="att", bufs=1)
        att = att_cm.__enter__()

        # gathered kv loads: one contiguous DMA per core (2.5KB/partition
        # runs) -- the transposed per-chunk pattern is 512B-run scatter and
        # runs descriptor-bound at ~1/3 wire speed
        for c in range(N_CORES):
            nc.sync.dma_start(kv_all[:, c, :], ag_out1[c])
        nc.sync.dma_start(
            kpe2[0:ROPE], agv1[:, :ROPE, 4, :].rearrange("c p s -> p c s"))
        nc.sync.dma_start(
            kpe2[ROPE:], agv1[:, :ROPE, 4, :].rearrange("c p s -> p c s"))

        # ======= phase 2: kv B-projection =======
        for nb in range(N_SB):
            sblk = slice(nb * 512, (nb + 1) * 512)
            c0 = nb * 2
            for h in range(HPC):
                acc = ps([P, 512], "qacc", "kn_acc", bufs=3)
                for k in range(N_KVC):
                    nc.tensor.matmul(
                        acc[:], wkvs[:, k, h * NOPE:(h + 1) * NOPE],
                        kv_all[:, c0:c0 + 2, k * SSH:(k + 1) * SSH],
                        start=(k == 0), stop=(k == N_KVC - 1))
                with nc.allow_low_precision(reason="flash operands are fp16"):
                    nc.vector.tensor_copy(kn[h][:, sblk], acc[:])
            for tsub in range(4):
                t_idx = nb * 4 + tsub
                acc = ps([P, HPC * VDIM], "qacc", "v_acc", bufs=3)
                o0 = (tsub % 2) * P
                for k in range(N_KVC):
                    nc.tensor.matmul(
                        acc[:],
                        kv_all[:, c0 + tsub // 2, k * SSH + o0:k * SSH + o0 + P],
                        wkvs[:, k, HPC * NOPE:],
                        start=(k == 0), stop=(k == N_KVC - 1))
                with nc.allow_low_precision(reason="flash operands are fp16"):
                    nc.vector.tensor_copy(vv[:, t_idx, :], acc[:])

        # ====== phases 3+4: q B-proj / rope / flash / o_proj, interleaved ===
        pending_norm = [None]
        pending_oproj = []

        def emit_norm(b, h, zac, oac, last=False):
            sblk = slice(b * 512, (b + 1) * 512)
            rz = work.tile([1, 512], F32, name="rz")
            nc.vector.reciprocal_approx_fast(rz[:], zac[:])
            rzr = work.tile([1, 512], F32R, name="rzr")
            with nc.allow_low_precision(reason="float32r rounding"):
                nc.scalar.copy(rzr[:], rz[:])
            bcz = ps([P, 512], "bcast", "bcz")
            nc.tensor.matmul(bcz[:], ones_row[:], rzr[:], start=True, stop=True)
            bczs = work.tile([P, 512], F32, name="bczs")
            if last:
                # tail: shortest chain, nothing left to overlap with
                nc.vector.tensor_copy(bczs[:], bcz[:])
                with nc.allow_low_precision(reason="fp16 o_proj operands"):
                    nc.vector.tensor_mul(ao[h][:, sblk], oac[:], bczs[:])
                return
            nc.scalar.copy(bczs[:], bcz[:])
            oraw = work.tile([P, 512], F32R, name="oraw")
            nc.scalar.copy(oraw[:], oac[:])
            with nc.allow_low_precision(reason="fp16 o_proj operands"):
                nc.gpsimd.tensor_mul(ao[h][:, sblk], oraw[:], bczs[:])

        def make_oproj(sc, nbc):
            def go():
                ssl = slice(sc * P, (sc + 1) * P)
                osl = slice(nbc * 512, (nbc + 1) * 512)
                acc = ps([P, 512], "qacc", "oo_acc", bufs=3)
                for kh in range(HPC):
                    nc.tensor.matmul(acc[:], ao[kh][:, ssl], wot[:, kh, osl],
                                     start=(kh == 0), stop=(kh == HPC - 1))
                ot = work.tile([P, 512], F16, name="ot", bufs=3)
                with nc.allow_low_precision(reason="fp16 output partials"):
                    nc.vector.tensor_copy(ot[:], acc[:])
                nc.sync.dma_start(yout[ssl, osl], ot[:])
            return go

        def make_oproj3_h0(g, sc, nbc):
            # h0 contribution of a block-3 o_proj group, staged to SBUF while
            # flash(3,1) still runs; the tail then only adds h1
            def go():
                ssl = slice(sc * P, (sc + 1) * P)
                osl = slice(nbc * 512, (nbc + 1) * 512)
                acc = ps([P, 512], "qacc", "oo_acc", bufs=3)
                nc.tensor.matmul(acc[:], ao[0][:, ssl], wot[:, 0, osl],
                                 start=True, stop=True)
                with nc.allow_low_precision(reason="fp16 output partials"):
                    nc.vector.tensor_copy(op3[:, g, :], acc[:])
            return go

        def make_oproj3_h1(g, sc, nbc):
            def go():
                ssl = slice(sc * P, (sc + 1) * P)
                osl = slice(nbc * 512, (nbc + 1) * 512)
                acc = ps([P, 512], "qacc", "oo_acc", bufs=3)
                nc.tensor.matmul(acc[:], ao[1][:, ssl], wot[:, 1, osl],
                                 start=True, stop=True)
                ot = work.tile([P, 512], F16, name="ot", bufs=3)
                with nc.allow_low_precision(reason="fp16 output partials"):
                    nc.vector.tensor_add(ot[:], acc[:], op3[:, g, :])
                nc.sync.dma_start(yout[ssl, osl], ot[:])
            return go

        def drain_norm(last=False):
            if pending_norm[0] is None:
                return
            b, h, zac, oac = pending_norm[0]
            pending_norm[0] = None
            emit_norm(b, h, zac, oac, last=last)
            if h == HPC - 1:
                g = 0
                for sc in range(4 * b, 4 * b + 4):
                    for nbc in range(N_SB):
                        if b == N_SB - 1:
                            pending_oproj.append(make_oproj3_h1(g, sc, nbc))
                            g += 1
                        else:
                            pending_oproj.append(make_oproj(sc, nbc))
            elif b == N_SB - 1:
                # norm(3,0) just emitted: stage the h0 partials into
                # flash(3,1)'s consume drains
                g = 0
                for sc in range(4 * b, 4 * b + 4):
                    for nbc in range(N_SB):
                        pending_oproj.append(make_oproj3_h0(g, sc, nbc))
                        g += 1

        def qproj_block(nb):
            sblk = slice(nb * 512, (nb + 1) * 512)
            c0 = nb * 2
            cqt = work.tile([P, 2, 512], F32R, name="cqt")
            nc.sync.dma_start(cqt[:, 0, :], cosq[:, sblk])
            nc.sync.dma_start(cqt[:, 1, :], ssinq[:, sblk])
            # deferred norm first (its broadcast rides under an o_proj group
            # when one is available), so the accumulator banks are clean
            q_blk = att.tile([P, 2, AG2_C * SSH], LAT_DT, name="qblk", bufs=2)
            nc.sync.dma_start(q_blk[:, 0, :], ag_out2[c0])
            nc.sync.dma_start(q_blk[:, 1, :], ag_out2[c0 + 1])
            if pending_oproj:
                pending_oproj.pop(0)()
            drain_norm()
            a_n = [ps([P, 512], "oacc", "qn0"), ps([P, 512], "oacc2", "qn1")]
            a_r = ps([P, 512], "qacc", "qr_acc", bufs=3)
            rbs = work.tile([P, 512], F32, name="rbs")
            crs = work.tile([P, 2, 512], F32R, name="crs")
            for k in range(N_QAC):
                qa = q_blk[:, :, k * SSH:(k + 1) * SSH]
                last = (k == N_QAC - 1)
                for h in range(HPC):
                    nc.tensor.matmul(a_n[h][:], wqs[:, k, h * NOPE:(h + 1) * NOPE],
                                     qa, start=(k == 0), stop=last)
                nc.tensor.matmul(a_r[:], wqs[:, k, HPC * NOPE:], qa,
                                 start=(k == 0), stop=last)
                if k == 2:
                    # rsqrt row -> broadcast -> premultiplied rope tables, all
                    # off the critical path while the k-loop streams
                    rr16 = work.tile([1, 512], F16, name="rr16")
                    roff = N_QAC * SSH
                    nc.sync.dma_start(rr16[:, 0:SSH],
                                      ag_out2[c0, 0:1, roff:roff + SSH])
                    nc.sync.dma_start(rr16[:, SSH:],
                                      ag_out2[c0 + 1, 0:1, roff:roff + SSH])
                    rrow = work.tile([1, 512], F32R, name="rrow")
                    nc.vector.tensor_copy(rrow[:], rr16[:])
                    rb = ps([P, 512], "bcast", "rb")
                    nc.tensor.matmul(rb[:], ones_row[:], rrow[:],
                                     start=True, stop=True)
                    nc.vector.tensor_copy(rbs[:], rb[:])
                    nc.vector.tensor_mul(crs[:, 0, :], cqt[:, 0, :], rbs[:])
                    nc.vector.tensor_mul(crs[:, 1, :], cqt[:, 1, :], rbs[:])
            # packed rope on both heads' q_pe (rsqrt premultiplied into crs)
            qp2s = work.tile([P, 512], F32R, name="qp2s")
            nc.scalar.copy(qp2s[:], a_r[:])
            swp2 = ps([P, 512], "qacc", "swp2", bufs=3)
            nc.tensor.matmul(swp2[:], psw[:], qp2s[:], start=True, stop=True)
            sw2 = work.tile([P, 512], F32R, name="sw2")
            nc.vector.tensor_mul(sw2[:], swp2[:], crs[:, 1, :])
            t2 = work.tile([P, 512], F32R, name="t2")
            nc.vector.tensor_mul(t2[:], qp2s[:], crs[:, 0, :])
            with nc.allow_low_precision(reason="flash operands are fp16"):
                nc.vector.tensor_add(qpb2[:, sblk], t2[:], sw2[:])
            for h in range(HPC):
                with nc.allow_low_precision(reason="flash operands are fp16"):
                    nc.vector.tensor_mul(qn[h][:, sblk], a_n[h][:], rbs[:])

        def flash_unit(b, h):
            sblk = slice(b * 512, (b + 1) * 512)
            n_tc = 4 * (b + 1)
            zac = ps([1, 512], "zacc" if h == 0 else "zacc2", "z_acc")
            oac = ps([P, 512], "oacc" if h == 0 else "oacc2", "o_acc")
            rsl = slice(ROPE * h, ROPE * (h + 1))
            exq = []

            def emit_scores(t):
                j = t - 4 * b
                w0 = 128 * j if j > 0 else 0
                cols = slice(w0, 512)
                tsl = slice(t * P, (t + 1) * P)
                sacc = ps([P, 512], "qacc", "s_acc", bufs=3)
                nc.tensor.matmul(sacc[:, cols], kn[h][:, tsl],
                                 qn[h][:, sblk][:, cols], start=True, stop=False)
                nc.tensor.matmul(sacc[:, cols], kpeA[rsl, tsl],
                                 qpb2[rsl, sblk][:, cols], start=False, stop=True)
                if j >= 0:
                    we = min(w0 + 128, 512)
                    nc.vector.tensor_add(sacc[:, w0:we], sacc[:, w0:we],
                                         tri[:, :we - w0])
                ex = epool.tile([P, 512], FLASH_DT, name="ex")
                nc.scalar.activation(ex[:, cols], sacc[:, cols], Exp,
                                     scale=SCALE, bias=negc[:])
                exq.append((ex, w0))

            def emit_consume(t):
                ex, w0 = exq.pop(0)
                cols = slice(w0, 512)
                nc.tensor.matmul(zac[:, cols], onesc_f[:], ex[:, cols],
                                 start=(t == 0), stop=(t == n_tc - 1))
                nc.tensor.matmul(oac[:, cols],
                                 vv[:, t, h * VDIM:(h + 1) * VDIM],
                                 ex[:, cols], start=(t == 0), stop=(t == n_tc - 1))
                if pending_oproj:
                    pending_oproj.pop(0)()

            depth = min(3, n_tc)
            for t in range(depth):
                emit_scores(t)
            drain_norm()
            for t in range(n_tc):
                if t + depth < n_tc:
                    emit_scores(t + depth)
                emit_consume(t)
            pending_norm[0] = (b, h, zac, oac)

        # qproj_block(b+1) is hoisted between flash(b,0) and flash(b,1) so its
        # vector-heavy epilogue hides under flash PE work instead of stalling
        # the next block's first score matmuls
        qproj_block(0)
        flash_unit(0, 0)
        for b in range(N_SB):
            if b + 1 < N_SB:
                qproj_block(b + 1)
            flash_unit(b, 1)
            if b + 1 < N_SB:
                flash_unit(b + 1, 0)
        drain_norm(last=True)
        while pending_oproj:
            pending_oproj.pop(0)()
        att_cm.__exit__(None, None, None)


_CACHED = None


def _get_program():
    global _CACHED
    if _CACHED is None:
        _CACHED = build_program()
    return _CACHED


def _host_prep(hidden_states, w_qkv_a, q_a_ln_w, w_q_b, w_kv_b, kv_a_ln_w, w_o,
               positions):
    f32 = np.float32
    hs = np.asarray(hidden_states, dtype=f32)
    w1m = np.asarray(w_qkv_a, dtype=f32)
    wqm = np.asarray(w_q_b, dtype=f32) * np.asarray(q_a_ln_w, f32)[None, :]
    wkvm = np.asarray(w_kv_b, dtype=f32) * np.asarray(kv_a_ln_w, f32)[None, :]
    wom = np.asarray(w_o, dtype=f32)

    # rope tables (interleaved / non-neox), matching the reference fp32 math
    pos = np.asarray(positions).astype(f32)
    inv_freq = (1.0 / (f32(THETA) ** (np.arange(0, ROPE, 2, dtype=f32) / f32(ROPE)))).astype(f32)
    fr = pos[None, :] * inv_freq[:, None]              # [32, S]
    cos = np.cos(fr).astype(f32)
    sin = np.sin(fr).astype(f32)
    cosT = np.repeat(cos, 2, axis=0)                   # [64, S]
    ssinT = np.empty((ROPE, SEQ), f32)
    ssinT[0::2] = -sin
    ssinT[1::2] = sin
    cosT2 = np.concatenate([cosT, cosT], axis=0)       # [128, S] both heads
    ssinT2 = np.concatenate([ssinT, ssinT], axis=0)
    psw = np.zeros((P, P), f32)                        # lhsT: out = psw.T @ x
    for i in range(0, P, 2):
        psw[i + 1, i] = 1.0                            # out[i]   = x[i+1]
        psw[i, i + 1] = 1.0                            # out[i+1] = x[i]
    tri = np.where(np.arange(P)[None, :] < np.arange(P)[:, None],
                   f32(NEG), f32(0.0))                 # [p, c]: NEG iff c < p

    hT = hs.T                                          # [I, S]
    # pad w_qkv_a^T out-dim 2112 -> 2176 (17*128); cols past 2112 are zero.
    # One 1MB DMA per output chunk m: w1l[m, p, k*128+j] = w1T[k*128+p, m*128+j]
    # so the (m, k) lhsT block is w1l[m][:, k*128:(k+1)*128].
    w1T = np.zeros((HIDDEN, LAT_C * P), f32)
    w1T[:, :QLR + KVLR + ROPE] = w1m.T
    w1l = np.ascontiguousarray(
        w1T.reshape(N_KC, P, LAT_C, P).transpose(2, 1, 0, 3).reshape(LAT_C, P, HIDDEN)).astype(np.float16)
    wq4 = wqm.reshape(H, QK, QLR)
    wkv4 = wkvm.reshape(H, NOPE + VDIM, KVLR)

    in_maps = []
    for c in range(N_CORES):
        ssl = slice(c * SSH, (c + 1) * SSH)
        h1 = np.ascontiguousarray(hT[:, ssl].reshape(N_KC, P, SSH).transpose(1, 0, 2)).astype(np.float16)
        # column order per k-chunk: [nope_h0 | nope_h1 | rope_h0 | rope_h1]
        wqc = wq4[HPC * c:HPC * (c + 1)]                            # [2, 192, 1536]
        wq_cols = np.concatenate([wqc[0, :NOPE], wqc[1, :NOPE],
                                  wqc[0, NOPE:], wqc[1, NOPE:]], axis=0)  # [384, QLR]
        wqT = wq_cols.T                                             # [QLR, 384]
        wql = np.ascontiguousarray(
            wqT.reshape(N_QAC, P, HPC * QK).transpose(1, 0, 2)).astype(np.float16)
        # column order per k-chunk: [kn_h0 | kn_h1 | v_h0 | v_h1]
        wkvc = wkv4[HPC * c:HPC * (c + 1)]                          # [2, 256, 512]
        wkv_cols = np.concatenate([wkvc[0, :NOPE], wkvc[1, :NOPE],
                                   wkvc[0, NOPE:], wkvc[1, NOPE:]], axis=0)  # [512, KVLR]
        wkvT = wkv_cols.T                                           # [KVLR, 512]
        wkvl = np.ascontiguousarray(
            wkvT.reshape(N_KVC, P, HPC * (NOPE + VDIM)).transpose(1, 0, 2)).astype(np.float16)
        woc = wom[:, HPC * VDIM * c:HPC * VDIM * (c + 1)].T          # [256, 2048]
        wol = np.ascontiguousarray(
            woc.reshape(HPC, P, HIDDEN).transpose(1, 0, 2)).astype(np.float16)
        in_maps.append({
            "h1": h1, "w1": w1l, "wq": wql, "wkv": wkvl, "wo": wol,
            "cosq": cosT2, "ssinq": ssinT2,
            "cosl": np.ascontiguousarray(cosT[:, ssl]),
            "ssinl": np.ascontiguousarray(ssinT[:, ssl]),
            "pswap": psw, "tri": tri,
            "onesc": np.ones((P, 1), f32),
            "onesr": np.ones((1, P), f32),
        })
    return in_maps


def kernel(**inputs):
    nc = _get_program()
    in_maps = _host_prep(**inputs)
    res = run_bass_kernel_spmd(nc, in_maps, list(range(N_CORES)))
    out = np.zeros((SEQ, HIDDEN), np.float64)
    for c in range(N_CORES):
        out += res.results[c]["y"].astype(np.float64)
    return out.astype(np.float32)


# revision 46
# speedup vs baseline: 1.2762x; 1.2762x over previous
"""DeepseekV2 MLA decoder-layer attention on 8 Trainium2 NeuronCores.

Distribution (tensor-parallel over heads, per the source hint):
  - A-projection (hidden @ w_qkv_a.T) is sequence-sharded: each core computes
    the fused low-rank latents for its 256-token shard.  The kv latents are
    rmsnorm'd locally; the q latents ship RAW and their rsqrt row rides in the
    same payload (folded into the q B-projection epilogue after the gather).
    ONE AllGather moves everything -- measurements show each collective has
    ~20us fixed latency on top of ~1.8us/MB wire time, so a single gather
    beats any split schedule.
  - B-projections, RoPE(q_pe), flash-style causal attention and o_proj are
    head-sharded: core c owns heads {2c, 2c+1}; its o_proj against the matching
    w_o column slice yields a partial [2048, 2048] output.
  - Unshard on host: output = sum of the 8 partials (RowParallel reduction).

Schedule notes:
  - Phase-1 sum-of-squares matmuls are emitted one chunk late so the PE never
    waits on the scalar engine's square of the chunk it just finished.
  - The q-rope B-projection is packed across the core's two heads into one
    128-row matmul; k_pe is duplicated into both partition halves so the
    flash rope-score matmuls run at base partition 64*h.
  - Flash normalization for unit i is deferred until unit i+1's first three
    score matmuls are queued; o_proj tile-groups are drained one-per-consume
    into the flash stream, spreading the output DMA across the tail.
  - Diagonal score tiles only compute the causal column range.
"""
import ml_dtypes
import numpy as np

import concourse.bass as bass
import concourse.mybir as mybir
import concourse.tile as tile
from concourse import bacc
from concourse.bass_utils import run_bass_kernel_spmd

HIDDEN = 2048
H = 16
NOPE = 128
ROPE = 64
VDIM = 128
QLR = 1536
KVLR = 512
QK = NOPE + ROPE            # 192
THETA = 10000.0
EPS = 1e-6
SEQ = 2048

N_CORES = 8
HPC = H // N_CORES          # 2 heads per core
SSH = SEQ // N_CORES        # 256-token shard
LAT_C = 17                  # latent chunks: 12 q_a + 4 kv_a + 1 (kpe, 64 rows)
P = 128

F32 = mybir.dt.float32
F32R = mybir.dt.float32r
BF16 = mybir.dt.bfloat16
F16 = mybir.dt.float16
F8 = mybir.dt.float8e4
FLASH_DT = F16              # dtype of q/k/v/exp inside flash attention
LAT_DT = F16                # dtype of the AG1 payload + kv B-proj operands
EXP_BIAS = -4.0             # exp(x*scale + EXP_BIAS): cancels in softmax ratio,
                            # keeps fp16 exp values in range
WQ_SCALE = 32.0             # w_q_b pre-scaled into fp8's sweet spot on host;
                            # divided back out in the exp scale

SCALE = float(QK) ** -0.5
NEG = -1.0e30

N_KC = HIDDEN // P          # 16
N_QAC = QLR // P            # 12
N_KVC = KVLR // P           # 4
N_SB = SEQ // 512           # 4 query blocks
N_SC = SEQ // P             # 16
AG2_C = N_QAC + 1           # raw q(12) + rsqrt-row(1), fp16


def build_program():
    nc = bacc.Bacc("TRN2", target_bir_lowering=False, debug=False,
                   num_devices=N_CORES)

    h1 = nc.dram_tensor("h1", [P, N_KC, SSH], F16, kind="ExternalInput")
    w1 = nc.dram_tensor("w1", [LAT_C, P, HIDDEN], F16, kind="ExternalInput")
    wq = nc.dram_tensor("wq", [P, N_QAC, HPC * QK], LAT_DT, kind="ExternalInput")
    wkv = nc.dram_tensor("wkv", [P, N_KVC, HPC * (NOPE + VDIM)], LAT_DT, kind="ExternalInput")
    wo = nc.dram_tensor("wo", [P, HPC, HIDDEN], F16, kind="ExternalInput")
    cosq = nc.dram_tensor("cosq", [P, SEQ], F32R, kind="ExternalInput")
    ssinq = nc.dram_tensor("ssinq", [P, SEQ], F32R, kind="ExternalInput")
    cosl = nc.dram_tensor("cosl", [ROPE, SSH], F32R, kind="ExternalInput")
    ssinl = nc.dram_tensor("ssinl", [ROPE, SSH], F32R, kind="ExternalInput")
    pswap = nc.dram_tensor("pswap", [P, P], F32R, kind="ExternalInput")
    tri_d = nc.dram_tensor("tri", [P, P], F32, kind="ExternalInput")
    onesc_d = nc.dram_tensor("onesc", [P, 1], F32R, kind="ExternalInput")
    onesr_d = nc.dram_tensor("onesr", [1, P], F32R, kind="ExternalInput")
    yout = nc.dram_tensor("y", [SEQ, HIDDEN], F16, kind="ExternalOutput")

    with tile.TileContext(nc) as tc:
        _emit(nc, tc, h1, w1, wq, wkv, wo, cosq, ssinq, cosl, ssinl, pswap,
              tri_d, onesc_d, onesr_d, yout)
    nc.compile()
    return nc


def _emit(nc, tc, h1, w1, wq, wkv, wo, cosq, ssinq, cosl, ssinl, pswap,
          tri_d, onesc_d, onesr_d, yout):
    Exp = mybir.ActivationFunctionType.Exp
    Sqrt = mybir.ActivationFunctionType.Sqrt
    rg = [list(range(N_CORES))]

    with tc.tile_pool(name="const", bufs=1) as const, \
         tc.tile_pool(name="work", bufs=2) as work, \
         tc.tile_pool(name="lstr", bufs=4) as lstr, \
         tc.tile_pool(name="epool", bufs=5) as epool, \
         tc.tile_pool(name="big", bufs=1) as big, \
         tc.tile_pool(name="psum", bufs=1, space="PSUM") as psum, \
         tc.tile_pool(name="dram", bufs=1, space="DRAM") as dram:

        # PSUM tags (8 banks): qacc x3, oacc, oacc2, zacc, zacc2, bcast
        def ps(shape, tag, name, bufs=None):
            return psum.tile(shape, F32, tag=tag, name=name, bufs=bufs)

        # ---- first critical DMAs: operands of the very first matmuls ----
        ph1_cm = tc.tile_pool(name="ph1", bufs=1)
        ph1 = ph1_cm.__enter__()
        hloc = ph1.tile([P, N_KC, SSH], F16)
        wt0 = ph1.tile([P, HIDDEN], F16, name="w1t", bufs=4)
        nc.sync.dma_start(wt0[:, 0:512], w1[16, :, 0:512])
        nc.sync.dma_start(hloc[:, 0, :], h1[:, 0, :])
        ones_col_t = const.tile([P, 1], F32R)
        nc.sync.dma_start(ones_col_t[:], onesc_d[:])
        ones_col = ones_col_t[:]
        nc.sync.dma_start(wt0[:, 512:], w1[16, :, 512:])
        for k in range(1, N_KC):
            nc.sync.dma_start(hloc[:, k, :], h1[:, k, :])
        ones_row_t = const.tile([1, P], F32R)
        nc.sync.dma_start(ones_row_t[:], onesr_d[:])
        ones_row = ones_row_t[:]
        psw = const.tile([P, P], F32R)
        nc.sync.dma_start(psw[:], pswap[:])
        cl = const.tile([ROPE, 2, SSH], F32R)
        nc.sync.dma_start(cl[:, 0, :], cosl[:])
        nc.sync.dma_start(cl[:, 1, :], ssinl[:])
        eps1 = const.tile([1, 1], F32)
        nc.vector.memset(eps1[:], EPS)
        negc = const.tile([P, 1], F32)
        nc.vector.memset(negc[:], EXP_BIAS)
        rpad = const.tile([P, SSH], F16)
        nc.vector.memset(rpad[:], 0.0)

        # ---- phase-3 tiles (declared early; DMAs staggered into phase 1) ----
        lat = ph1.tile([P, 5, SSH], F32R)       # kv chunks 12..15 + kpe
        latf = ph1.tile([P, 5, SSH], LAT_DT)    # normalized kv + kpe, fp16
        latq8 = ph1.tile([P, N_QAC, SSH], LAT_DT)  # raw q latents, fp16
        wqs = big.tile([P, N_QAC, HPC * QK], LAT_DT)
        wkvs = big.tile([P, N_KVC, HPC * (NOPE + VDIM)], LAT_DT)
        wot = big.tile([P, HPC, HIDDEN], F16)
        tri = const.tile([P, P], F32)
        kn = [big.tile([P, SEQ], FLASH_DT, name=f"kn{h}") for h in range(HPC)]
        qn = [big.tile([P, SEQ], FLASH_DT, name=f"qn{h}") for h in range(HPC)]
        qpb2 = big.tile([P, SEQ], FLASH_DT)
        kpe2 = big.tile([P, N_CORES, SSH], LAT_DT)
        kv_all = big.tile([P, N_CORES, 5 * SSH], LAT_DT)
        vv = big.tile([P, N_SC, HPC * VDIM], FLASH_DT)
        ao = [big.tile([P, SEQ], F16, name=f"ao{h}") for h in range(HPC)]
        onesc_f = const.tile([P, 1], FLASH_DT)
        with nc.allow_low_precision(reason="ones vector"):
            nc.vector.tensor_copy(onesc_f[:], ones_col)

        ss_q = ps([1, SSH], "zacc", "ss_q")
        ss_kv = ps([1, SSH], "zacc2", "ss_kv")

        def kv_norm_chain():
            rt = work.tile([1, SSH], F32, name="rtkv", bufs=1)
            nc.scalar.activation(rt[:], ss_kv[:], Sqrt, bias=eps1[:], scale=1.0 / KVLR)
            ri = work.tile([1, SSH], F32, name="rikv", bufs=1)
            nc.vector.reciprocal_approx_fast(ri[:], rt[:])
            rir = work.tile([1, SSH], F32R, name="rikvr", bufs=1)
            with nc.allow_low_precision(reason="float32r rounding"):
                nc.scalar.copy(rir[:], ri[:])
            bc = ps([P, SSH], "bcast", "bckv")
            nc.tensor.matmul(bc[:], ones_row[:], rir[:], start=True, stop=True)
            bcs = work.tile([P, SSH], F32, name="bcskv", bufs=1)
            nc.vector.tensor_copy(bcs[:], bc[:])
            for m in range(N_KVC):
                with nc.allow_low_precision(reason="fp16 AllGather payload"):
                    nc.vector.tensor_mul(latf[:, m, :], lat[:, m, :], bcs[:])

        ag_in1 = dram.tile([P, 5 * SSH], LAT_DT)
        ag_out1 = dram.tile([N_CORES, P, 5 * SSH], LAT_DT, addr_space="Shared")
        ag_in2 = dram.tile([P, AG2_C * SSH], LAT_DT)
        ag_out2 = dram.tile([N_CORES, P, AG2_C * SSH], LAT_DT, addr_space="Shared")

        # ================= phase 1: local A-proj =================
        # kv chunks first (12..15), then kpe (16), then raw q (0..11).
        # ss matmuls are deferred one chunk so the PE never waits on the
        # scalar square of the chunk it just finished.
        m_order = [16] + list(range(N_QAC, 16)) + list(range(N_QAC))
        pending_ss = [None]

        def drain_ss():
            if pending_ss[0] is not None:
                m_, sq_ = pending_ss[0]
                pending_ss[0] = None
                tgt = ss_q if m_ < N_QAC else ss_kv
                nc.tensor.matmul(tgt[:], ones_col[:], sq_[:],
                                 start=(m_ == 0) or (m_ == N_QAC),
                                 stop=(m_ == N_QAC - 1) or (m_ == 15))

        for m in m_order:
            acc = ps([P, SSH], "qacc", "a_acc", bufs=3)
            if m == m_order[0]:
                wt = wt0
            else:
                wt = ph1.tile([P, HIDDEN], F16, name="w1t", bufs=4)
                nc.sync.dma_start(wt[:], w1[m])
            for k in range(N_KC):
                nc.tensor.matmul(acc[:], wt[:, k * P:(k + 1) * P], hloc[:, k, :],
                                 start=(k == 0), stop=(k == N_KC - 1))
                if k == 2:
                    drain_ss()
            if m >= N_QAC:
                nc.vector.tensor_copy(lat[:, m - N_QAC, :], acc[:])
            else:
                with nc.allow_low_precision(reason="raw q latents ship fp16"):
                    nc.vector.tensor_copy(latq8[:, m, :], acc[:])
            if m < 16:
                sq = work.tile([P, SSH], F32R, name="sq", bufs=3)
                nc.scalar.square(sq[:], acc[:])
                if m == 15:
                    # emit immediately: the kv rmsnorm and AG1 trigger follow
                    drain_ss()
                    pending_ss[0] = (m, sq)
                    drain_ss()
                else:
                    pending_ss[0] = (m, sq)

            if m == 7:
                nc.sync.dma_start(wkvs[:], wkv[:])
            if m in (8, 9, 10):
                # phase-3 weight prefetch, late in phase 1 so the critical
                # early weight stream is undisturbed
                j = m - 8
                nc.sync.dma_start(wqs[:, 4 * j:4 * (j + 1), :],
                                  wq[:, 4 * j:4 * (j + 1), :])
            if m == 11:
                nc.sync.dma_start(wot[:], wo[:])
                nc.sync.dma_start(tri[:], tri_d[:])
            if m == 16:
                # rope on local k_pe (chunk 16 = lat idx 4, rows 0:64)
                swp = ps([ROPE, SSH], "qacc", "swp", bufs=3)
                nc.tensor.matmul(swp[:], psw[:ROPE, :ROPE], lat[:ROPE, 4, :],
                                 start=True, stop=True)
                sws = work.tile([ROPE, SSH], F32R, name="sws", bufs=1)
                nc.vector.tensor_mul(sws[:], swp[:], cl[:, 1, :])
                t1 = work.tile([ROPE, SSH], F32R, name="t1", bufs=1)
                nc.vector.tensor_mul(t1[:], lat[:ROPE, 4, :], cl[:, 0, :])
                with nc.allow_low_precision(reason="fp16 AllGather payload"):
                    nc.vector.tensor_add(latf[:ROPE, 4, :], t1[:], sws[:])
                    nc.vector.memset(latf[ROPE:, 4, :], 0.0)
            if m == 15:
                kv_norm_chain()
                # ---- AG1 (kv + kpe latents) overlaps the q-chunk matmuls
                nc.sync.dma_start(
                    ag_in1[:], latf[:].rearrange("p m s -> p (m s)"))
                nc.gpsimd.collective_compute(
                    "AllGather", mybir.AluOpType.bypass, replica_groups=rg,
                    ins=[ag_in1.opt()], outs=[ag_out1.opt()],
                )
            if m == 5:
                nc.sync.dma_start(
                    ag_in2[:, :6 * SSH],
                    latq8[:, 0:6, :].rearrange("p m s -> p (m s)"))

        drain_ss()
        # q rsqrt row (normalization folded in after the gather)
        rt_q = work.tile([1, SSH], F32, name="rtq", bufs=1)
        nc.scalar.activation(rt_q[:], ss_q[:], Sqrt, bias=eps1[:], scale=1.0 / QLR)
        ri_q = work.tile([1, SSH], F32, name="riq", bufs=1)
        nc.vector.reciprocal_approx_fast(ri_q[:], rt_q[:])
        with nc.allow_low_precision(reason="fp16 AllGather payload"):
            nc.vector.tensor_copy(rpad[0:1, :], ri_q[:])
        nc.sync.dma_start(
            ag_in2[:, 6 * SSH:12 * SSH],
            latq8[:, 6:12, :].rearrange("p m s -> p (m s)"))
        nc.sync.dma_start(ag_in2[:, 12 * SSH:], rpad[:])
        nc.gpsimd.collective_compute(
            "AllGather", mybir.AluOpType.bypass, replica_groups=rg,
            ins=[ag_in2.opt()], outs=[ag_out2.opt()],
        )
        # gathered views: agv1 idx 0..3 = kv, 4 = kpe; agv2 0..11 = raw q,
        # 12 = rsqrt row
        agv1 = ag_out1[:].rearrange("c p (m s) -> c p m s", m=5)
        agv2 = ag_out2[:].rearrange("c p (m s) -> c p m s", m=AG2_C)
        kpeA = kpe2[:].rearrange("p c s -> p (c s)")
        ph1_cm.__exit__(None, None, None)
        att_cm = tc.tile_pool(name="att", bufs=1)
        att = att_cm.__enter__()

        # gathered kv loads: one contiguous DMA per core (2.5KB/partition
        # runs) -- the transposed per-chunk pattern is 512B-run scatter and
        # runs descriptor-bound at ~1/3 wire speed
        for c in range(N_CORES):
            nc.sync.dma_start(kv_all[:, c, :], ag_out1[c])
        nc.sync.dma_start(
            kpe2[0:ROPE], agv1[:, :ROPE, 4, :].rearrange("c p s -> p c s"))
        nc.sync.dma_start(
            kpe2[ROPE:], agv1[:, :ROPE, 4, :].rearrange("c p s -> p c s"))

        # ======= phase 2: kv B-projection =======
        for nb in range(N_SB):
            sblk = slice(nb * 512, (nb + 1) * 512)
            c0 = nb * 2
            for h in range(HPC):
                acc = ps([P, 512], "qacc", "kn_acc", bufs=3)
                for k in range(N_KVC):
                    nc.tensor.matmul(
                        acc[:], wkvs[:, k, h * NOPE:(h + 1) * NOPE],
                        kv_all[:, c0:c0 + 2, k * SSH:(k + 1) * SSH],
                        start=(k == 0), stop=(k == N_KVC - 1))
                with nc.allow_low_precision(reason="flash operands are fp16"):
                    nc.vector.tensor_copy(kn[h][:, sblk], acc[:])
            for tsub in range(4):
                t_idx = nb * 4 + tsub
                acc = ps([P, HPC * VDIM], "qacc", "v_acc", bufs=3)
                o0 = (tsub % 2) * P
                for k in range(N_KVC):
                    nc.tensor.matmul(
                        acc[:],
                        kv_all[:, c0 + tsub // 2, k * SSH + o0:k * SSH + o0 + P],
                        wkvs[:, k, HPC * NOPE:],
                        start=(k == 0), stop=(k == N_KVC - 1))
                with nc.allow_low_precision(reason="flash operands are fp16"):
                    nc.vector.tensor_copy(vv[:, t_idx, :], acc[:])

        # ====== phases 3+4: q B-proj / rope / flash / o_proj, interleaved ===
        pending_norm = [None]
        pending_oproj = []

        def emit_norm(b, h, zac, oac, last=False):
            sblk = slice(b * 512, (b + 1) * 512)
            rz = work.tile([1, 512], F32, name="rz")
            nc.vector.reciprocal_approx_fast(rz[:], zac[:])
            rzr = work.tile([1, 512], F32R, name="rzr")
            with nc.allow_low_precision(reason="float32r rounding"):
                nc.scalar.copy(rzr[:], rz[:])
            bcz = ps([P, 512], "bcast", "bcz")
            nc.tensor.matmul(bcz[:], ones_row[:], rzr[:], start=True, stop=True)
            bczs = work.tile([P, 512], F32, name="bczs")
            if last:
                # tail: shortest chain, nothing left to overlap with
                nc.vector.tensor_copy(bczs[:], bcz[:])
                with nc.allow_low_precision(reason="fp16 o_proj operands"):
                    nc.vector.tensor_mul(ao[h][:, sblk], oac[:], bczs[:])
                return
            nc.scalar.copy(bczs[:], bcz[:])
            oraw = work.tile([P, 512], F32R, name="oraw")
            nc.scalar.copy(oraw[:], oac[:])
            with nc.allow_low_precision(reason="fp16 o_proj operands"):
                nc.gpsimd.tensor_mul(ao[h][:, sblk], oraw[:], bczs[:])

        def make_oproj(sc, nbc):
            def go():
                ssl = slice(sc * P, (sc + 1) * P)
                osl = slice(nbc * 512, (nbc + 1) * 512)
                acc = ps([P, 512], "qacc", "oo_acc", bufs=3)
                for kh in range(HPC):
                    nc.tensor.matmul(acc[:], ao[kh][:, ssl], wot[:, kh, osl],
                                     start=(kh == 0), stop=(kh == HPC - 1))
                ot = work.tile([P, 512], F16, name="ot", bufs=3)
                with nc.allow_low_precision(reason="fp16 output partials"):
                    nc.vector.tensor_copy(ot[:], acc[:])
                nc.sync.dma_start(yout[ssl, osl], ot[:])
            return go

        def drain_norm(last=False):
            if pending_norm[0] is None:
                return
            b, h, zac, oac = pending_norm[0]
            pending_norm[0] = None
            emit_norm(b, h, zac, oac, last=last)
            if h == HPC - 1:
                for sc in range(4 * b, 4 * b + 4):
                    for nbc in range(N_SB):
                        pending_oproj.append(make_oproj(sc, nbc))

        def qproj_block(nb):
            sblk = slice(nb * 512, (nb + 1) * 512)
            c0 = nb * 2
            cqt = work.tile([P, 2, 512], F32R, name="cqt")
            nc.sync.dma_start(cqt[:, 0, :], cosq[:, sblk])
            nc.sync.dma_start(cqt[:, 1, :], ssinq[:, sblk])
            # deferred norm first (its broadcast rides under an o_proj group
            # when one is available), so the accumulator banks are clean
            q_blk = att.tile([P, 2, AG2_C * SSH], LAT_DT, name="qblk", bufs=2)
            nc.sync.dma_start(q_blk[:, 0, :], ag_out2[c0])
            nc.sync.dma_start(q_blk[:, 1, :], ag_out2[c0 + 1])
            if pending_oproj:
                pending_oproj.pop(0)()
            drain_norm()
            a_n = [ps([P, 512], "oacc", "qn0"), ps([P, 512], "oacc2", "qn1")]
            a_r = ps([P, 512], "qacc", "qr_acc", bufs=3)
            rbs = work.tile([P, 512], F32, name="rbs")
            crs = work.tile([P, 2, 512], F32R, name="crs")
            for k in range(N_QAC):
                qa = q_blk[:, :, k * SSH:(k + 1) * SSH]
                last = (k == N_QAC - 1)
                for h in range(HPC):
                    nc.tensor.matmul(a_n[h][:], wqs[:, k, h * NOPE:(h + 1) * NOPE],
                                     qa, start=(k == 0), stop=last)
                nc.tensor.matmul(a_r[:], wqs[:, k, HPC * NOPE:], qa,
                                 start=(k == 0), stop=last)
                if k == 2:
                    # rsqrt row -> broadcast -> premultiplied rope tables, all
                    # off the critical path while the k-loop streams
                    rr16 = work.tile([1, 512], F16, name="rr16")
                    roff = N_QAC * SSH
                    nc.sync.dma_start(rr16[:, 0:SSH],
                                      ag_out2[c0, 0:1, roff:roff + SSH])
                    nc.sync.dma_start(rr16[:, SSH:],
                                      ag_out2[c0 + 1, 0:1, roff:roff + SSH])
                    rrow = work.tile([1, 512], F32R, name="rrow")
                    nc.vector.tensor_copy(rrow[:], rr16[:])
                    rb = ps([P, 512], "bcast", "rb")
                    nc.tensor.matmul(rb[:], ones_row[:], rrow[:],
                                     start=True, stop=True)
                    nc.vector.tensor_copy(rbs[:], rb[:])
                    nc.vector.tensor_mul(crs[:, 0, :], cqt[:, 0, :], rbs[:])
                    nc.vector.tensor_mul(crs[:, 1, :], cqt[:, 1, :], rbs[:])
            # packed rope on both heads' q_pe (rsqrt premultiplied into crs)
            qp2s = work.tile([P, 512], F32R, name="qp2s")
            nc.scalar.copy(qp2s[:], a_r[:])
            swp2 = ps([P, 512], "qacc", "swp2", bufs=3)
            nc.tensor.matmul(swp2[:], psw[:], qp2s[:], start=True, stop=True)
            sw2 = work.tile([P, 512], F32R, name="sw2")
            nc.vector.tensor_mul(sw2[:], swp2[:], crs[:, 1, :])
            t2 = work.tile([P, 512], F32R, name="t2")
            nc.vector.tensor_mul(t2[:], qp2s[:], crs[:, 0, :])
            with nc.allow_low_precision(reason="flash operands are fp16"):
                nc.vector.tensor_add(qpb2[:, sblk], t2[:], sw2[:])
            for h in range(HPC):
                with nc.allow_low_precision(reason="flash operands are fp16"):
                    nc.vector.tensor_mul(qn[h][:, sblk], a_n[h][:], rbs[:])

        def flash_unit(b, h):
            sblk = slice(b * 512, (b + 1) * 512)
            n_tc = 4 * (b + 1)
            zac = ps([1, 512], "zacc" if h == 0 else "zacc2", "z_acc")
            oac = ps([P, 512], "oacc" if h == 0 else "oacc2", "o_acc")
            rsl = slice(ROPE * h, ROPE * (h + 1))
            exq = []

            def emit_scores(t):
                j = t - 4 * b
                w0 = 128 * j if j > 0 else 0
                cols = slice(w0, 512)
                tsl = slice(t * P, (t + 1) * P)
                sacc = ps([P, 512], "qacc", "s_acc", bufs=3)
                nc.tensor.matmul(sacc[:, cols], kn[h][:, tsl],
                                 qn[h][:, sblk][:, cols], start=True, stop=False)
                nc.tensor.matmul(sacc[:, cols], kpeA[rsl, tsl],
                                 qpb2[rsl, sblk][:, cols], start=False, stop=True)
                if j >= 0:
                    we = min(w0 + 128, 512)
                    nc.vector.tensor_add(sacc[:, w0:we], sacc[:, w0:we],
                                         tri[:, :we - w0])
                ex = epool.tile([P, 512], FLASH_DT, name="ex")
                nc.scalar.activation(ex[:, cols], sacc[:, cols], Exp,
                                     scale=SCALE, bias=negc[:])
                exq.append((ex, w0))

            def emit_consume(t):
                ex, w0 = exq.pop(0)
                cols = slice(w0, 512)
                nc.tensor.matmul(zac[:, cols], onesc_f[:], ex[:, cols],
                                 start=(t == 0), stop=(t == n_tc - 1))
                nc.tensor.matmul(oac[:, cols],
                                 vv[:, t, h * VDIM:(h + 1) * VDIM],
                                 ex[:, cols], start=(t == 0), stop=(t == n_tc - 1))
                if pending_oproj:
                    pending_oproj.pop(0)()

            depth = min(3, n_tc)
            for t in range(depth):
                emit_scores(t)
            drain_norm()
            for t in range(n_tc):
                if t + depth < n_tc:
                    emit_scores(t + depth)
                emit_consume(t)
            pending_norm[0] = (b, h, zac, oac)

        # qproj_block(b+1) is hoisted between flash(b,0) and flash(b,1) so its
        # vector-heavy epilogue hides under flash PE work instead of stalling
        # the next block's first score matmuls
        qproj_block(0)
        flash_unit(0, 0)
        for b in range(N_SB):
            if b + 1 < N_SB:
                qproj_block(b + 1)
            flash_unit(b, 1)
            if b + 1 < N_SB:
                flash_unit(b + 1, 0)
        drain_norm(last=True)
        while pending_oproj:
            pending_oproj.pop(0)()
        att_cm.__exit__(None, None, None)


_CACHED = None


def _get_program():
    global _CACHED
    if _CACHED is None:
        _CACHED = build_program()
    return _CACHED


def _host_prep(hidden_states, w_qkv_a, q_a_ln_w, w_q_b, w_kv_b, kv_a_ln_w, w_o,
               positions):
    f32 = np.float32
    hs = np.asarray(hidden_states, dtype=f32)
    w1m = np.asarray(w_qkv_a, dtype=f32)
    wqm = np.asarray(w_q_b, dtype=f32) * np.asarray(q_a_ln_w, f32)[None, :]
    wkvm = np.asarray(w_kv_b, dtype=f32) * np.asarray(kv_a_ln_w, f32)[None, :]
    wom = np.asarray(w_o, dtype=f32)

    # rope tables (interleaved / non-neox), matching the reference fp32 math
    pos = np.asarray(positions).astype(f32)
    inv_freq = (1.0 / (f32(THETA) ** (np.arange(0, ROPE, 2, dtype=f32) / f32(ROPE)))).astype(f32)
    fr = pos[None, :] * inv_freq[:, None]              # [32, S]
    cos = np.cos(fr).astype(f32)
    sin = np.sin(fr).astype(f32)
    cosT = np.repeat(cos, 2, axis=0)                   # [64, S]
    ssinT = np.empty((ROPE, SEQ), f32)
    ssinT[0::2] = -sin
    ssinT[1::2] = sin
    cosT2 = np.concatenate([cosT, cosT], axis=0)       # [128, S] both heads
    ssinT2 = np.concatenate([ssinT, ssinT], axis=0)
    psw = np.zeros((P, P), f32)                        # lhsT: out = psw.T @ x
    for i in range(0, P, 2):
        psw[i + 1, i] = 1.0                            # out[i]   = x[i+1]
        psw[i, i + 1] = 1.0                            # out[i+1] = x[i]
    tri = np.where(np.arange(P)[None, :] < np.arange(P)[:, None],
                   f32(NEG), f32(0.0))                 # [p, c]: NEG iff c < p

    hT = hs.T                                          # [I, S]
    # pad w_qkv_a^T out-dim 2112 -> 2176 (17*128); cols past 2112 are zero.
    # One 1MB DMA per output chunk m: w1l[m, p, k*128+j] = w1T[k*128+p, m*128+j]
    # so the (m, k) lhsT block is w1l[m][:, k*128:(k+1)*128].
    w1T = np.zeros((HIDDEN, LAT_C * P), f32)
    w1T[:, :QLR + KVLR + ROPE] = w1m.T
    w1l = np.ascontiguousarray(
        w1T.reshape(N_KC, P, LAT_C, P).transpose(2, 1, 0, 3).reshape(LAT_C, P, HIDDEN)).astype(np.float16)
    wq4 = wqm.reshape(H, QK, QLR)
    wkv4 = wkvm.reshape(H, NOPE + VDIM, KVLR)

    in_maps = []
    for c in range(N_CORES):
        ssl = slice(c * SSH, (c + 1) * SSH)
        h1 = np.ascontiguousarray(hT[:, ssl].reshape(N_KC, P, SSH).transpose(1, 0, 2)).astype(np.float16)
        # column order per k-chunk: [nope_h0 | nope_h1 | rope_h0 | rope_h1]
        wqc = wq4[HPC * c:HPC * (c + 1)]                            # [2, 192, 1536]
        wq_cols = np.concatenate([wqc[0, :NOPE], wqc[1, :NOPE],
                                  wqc[0, NOPE:], wqc[1, NOPE:]], axis=0)  # [384, QLR]
        wqT = wq_cols.T                                             # [QLR, 384]
        wql = np.ascontiguousarray(
            wqT.reshape(N_QAC, P, HPC * QK).transpose(1, 0, 2)).astype(np.float16)
        # column order per k-chunk: [kn_h0 | kn_h1 | v_h0 | v_h1]
        wkvc = wkv4[HPC * c:HPC * (c + 1)]                          # [2, 256, 512]
        wkv_cols = np.concatenate([wkvc[0, :NOPE], wkvc[1, :NOPE],
                                   wkvc[0, NOPE:], wkvc[1, NOPE:]], axis=0)  # [512, KVLR]
        wkvT = wkv_cols.T                                           # [KVLR, 512]
        wkvl = np.ascontiguousarray(
            wkvT.reshape(N_KVC, P, HPC * (NOPE + VDIM)).transpose(1, 0, 2)).astype(np.float16)
        woc = wom[:, HPC * VDIM * c:HPC * VDIM * (c + 1)].T          # [256, 2048]
        wol = np.ascontiguousarray(
            woc.reshape(HPC, P, HIDDEN).transpose(1, 0, 2)).astype(np.float16)
        in_maps.append({
            "h1": h1, "w1": w1l, "wq": wql, "wkv": wkvl, "wo": wol,
            "cosq": cosT2, "ssinq": ssinT2,
            "cosl": np.ascontiguousarray(cosT[:, ssl]),
            "ssinl": np.ascontiguousarray(ssinT[:, ssl]),
            "pswap": psw, "tri": tri,
            "onesc": np.ones((P, 1), f32),
            "onesr": np.ones((1, P), f32),
        })
    return in_maps


def kernel(**inputs):
    nc = _get_program()
    in_maps = _host_prep(**inputs)
    res = run_bass_kernel_spmd(nc, in_maps, list(range(N_CORES)))
    out = np.zeros((SEQ, HIDDEN), np.float64)
    for c in range(N_CORES):
        out += res.results[c]["y"].astype(np.float64)
    return out.astype(np.float32)
